# revision 3
# baseline (speedup 1.0000x reference)
"""Trainium2 Bass kernel for BilinearPoolingFusion.

Math (fp32 reference):
    x2m = mean(x2, axis=1)                  # [b, d2]
    t   = einsum('oij,bj->boi', W, x2m)     # [b, d3, d1]
    out = einsum('bli,boi->blo', x1, t) + b # [b, l1, d3]

Distribution: tensor-parallel over d3 (output channels) across 8 cores —
W is read exactly once globally (it dominates traffic at 537MB); each core
computes out[:, :, o_shard] for its 64 channels. x1/x2 are replicated.

Per-core device pipeline:
  1. x2 mean: DVE accumulate l2-tiles + PE ones-matmul partition reduce
     -> x2mT [j(part), b] (fp32 exact, rounded to f32r for the PE).
  2. t = W . x2m: stream host-pretransposed W^T [j, o, i] through the PE
     as the moving operand in float32r (full rate at N=512);
     psum [b, i(512)] per o -> PE-transpose to tT [i(part), o] per b.
  3. out = x1 @ t: stationary tT [i, o-group], moving x1T [i, l] (f32r),
     psum [o, l] -> +bias -> DMA out as [b, o, l] (host de-transposes).

float32r is tf32-like (~1.6e-4 rel err) at 4x the fp32 matmul rate.
"""

import atexit
from contextlib import ExitStack

import numpy as np

import concourse.bass as bass
import concourse.tile as tile
from concourse import bacc, mybir
from concourse.bass_utils import run_bass_kernel_spmd
from concourse.masks import make_identity

F32 = mybir.dt.float32
F32R = mybir.dt.float32r

N_CORES = 8
NB = 8          # batch
L1 = 1024       # x1 sequence length
L2 = 1024       # x2 sequence length (mean dim)
D = 512         # d1 = d2 = d3
OSH = D // N_CORES   # 64 output channels per core
JC = D // 128        # 4 contraction chunks over j
ISEG = D // 128      # 4 segments over i
OG = 32              # step-3 o-group size (2 groups per core)
P = 128


def _emit(nc):
    wt = nc.dram_tensor("wt", [D, OSH, D], F32R, kind="ExternalInput").ap()    # [j, o, i]
    x1t = nc.dram_tensor("x1t", [NB, D, L1], F32R, kind="ExternalInput").ap()  # [b, i, l]
    x2 = nc.dram_tensor("x2", [NB, L2, D], F32, kind="ExternalInput").ap()     # [b, l2, j]
    bias = nc.dram_tensor("bias", [OSH, 1], F32, kind="ExternalInput").ap()
    out = nc.dram_tensor("out", [NB, OSH, L1], F32, kind="ExternalOutput").ap()  # [b, o, l]

    with tile.TileContext(nc) as tc, ExitStack() as ctx:
        consts = ctx.enter_context(tc.tile_pool(name="consts", bufs=1))
        x2p = ctx.enter_context(tc.tile_pool(name="x2p", bufs=2))
        accp = ctx.enter_context(tc.tile_pool(name="accp", bufs=2))
        wp = ctx.enter_context(tc.tile_pool(name="wp", bufs=3))
        tp = ctx.enter_context(tc.tile_pool(name="tp", bufs=4))
        x1p = ctx.enter_context(tc.tile_pool(name="x1p", bufs=1))
        tTp = ctx.enter_context(tc.tile_pool(name="tTp", bufs=1))
        outp = ctx.enter_context(tc.tile_pool(name="outp", bufs=2))
        ps_t = ctx.enter_context(tc.tile_pool(name="ps_t", bufs=3, space="PSUM"))
        ps_tr = ctx.enter_context(tc.tile_pool(name="ps_tr", bufs=2, space="PSUM"))
        ps_o = ctx.enter_context(tc.tile_pool(name="ps_o", bufs=3, space="PSUM"))

        ident_f = consts.tile([P, P], F32)
        make_identity(nc, ident_f[:])
        ident = consts.tile([P, P], F32R)
        nc.vector.tensor_copy(ident[:], ident_f[:])
        ones = consts.tile([P, 1], F32)
        nc.vector.memset(ones[:], 1.0 / L2)
        biast = consts.tile([OSH, 1], F32)
        nc.sync.dma_start(biast[:], bias[:])
        # x2mT columns: (jc, b) -> jc*NB + b
        x2mT = consts.tile([P, JC * NB], F32R)

        # ---- phase 1: x2 mean -> x2mT [j(part), b] ----
        for b in range(NB):
            acc = accp.tile([P, D], F32)
            for half in range(2):
                xt = x2p.tile([P, 4, D], F32)
                src = x2[b, half * 512:(half + 1) * 512, :].rearrange(
                    "(t p) d -> p t d", p=P
                )
                nc.sync.dma_start(xt[:], src)
                for k in range(4):
                    if half == 0 and k == 0:
                        nc.vector.tensor_copy(acc[:], xt[:, 0, :])
                    else:
                        nc.vector.tensor_add(acc[:], acc[:], xt[:, k, :])
            for c in range(JC):
                pm = ps_tr.tile([P, NB], F32, tag="tr")
                nc.tensor.matmul(
                    pm[:, 0:1], acc[:, c * P:(c + 1) * P], ones[:],
                    start=True, stop=True,
                )
                nc.vector.tensor_copy(
                    x2mT[:, c * NB + b:c * NB + b + 1], pm[:, 0:1]
                )

        # ---- phases 2 + 3, interleaved by o-group ----
        x1sb = None
        for og in range(2):
            tT = tTp.tile([P, ISEG, OG, NB], F32R, tag=f"tT{og}")
            for o_loc in range(OG):
                o = og * OG + o_loc
                wt_t = wp.tile([P, JC, D], F32R)
                nc.sync.dma_start(
                    wt_t[:], wt[:, o, :].rearrange("(c p) i -> p c i", p=P)
                )
                ps = ps_t.tile([NB, D], F32)
                for jc in range(JC):
                    nc.tensor.matmul(
                        ps[:], x2mT[:, jc * NB:(jc + 1) * NB], wt_t[:, jc, :],
                        start=(jc == 0), stop=(jc == JC - 1),
                    )
                tsb = tp.tile([NB, D], F32R)
                nc.scalar.copy(tsb[:], ps[:])
                for c in range(ISEG):
                    ptr = ps_tr.tile([P, NB], F32R, tag="tr")
                    nc.tensor.transpose(
                        ptr[:], tsb[:, c * P:(c + 1) * P], ident[:NB, :NB]
                    )
                    nc.vector.tensor_copy(tT[:, c, o_loc, :], ptr[:])

            if og == 0:
                # emitted after the first W wave so x2/W DMAs get the queues first
                x1sb = x1p.tile([P, NB, ISEG, L1], F32R)
                for b in range(NB):
                    nc.sync.dma_start(
                        x1sb[:, b], x1t[b].rearrange("(s p) l -> p s l", p=P)
                    )

            for b in range(NB):
                osb = outp.tile([OG, L1], F32)
                for lh in range(2):
                    po = ps_o.tile([OG, 512], F32)
                    for s in range(ISEG):
                        nc.tensor.matmul(
                            po[:], tT[:, s, :, b],
                            x1sb[:, b, s, lh * 512:(lh + 1) * 512],
                            start=(s == 0), stop=(s == ISEG - 1),
                        )
                    nc.scalar.add(
                        osb[:, lh * 512:(lh + 1) * 512], po[:],
                        add=biast[og * OG:(og + 1) * OG, :],
                    )
                nc.sync.dma_start(out[b, og * OG:(og + 1) * OG, :], osb[:])


_nc_cache = None


def _get_nc():
    global _nc_cache
    if _nc_cache is None:
        nc = bacc.Bacc(
            "TRN2", target_bir_lowering=False, debug=False, num_devices=N_CORES
        )
        _emit(nc)
        nc.compile()
        _nc_cache = nc
    return _nc_cache


def kernel(x1, x2, W, b):
    x1 = np.ascontiguousarray(x1, dtype=np.float32)
    x2 = np.ascontiguousarray(x2, dtype=np.float32)
    W = np.ascontiguousarray(W, dtype=np.float32)
    b = np.ascontiguousarray(b, dtype=np.float32)

    nc = _get_nc()
    x1t = np.ascontiguousarray(np.transpose(x1, (0, 2, 1)))  # [b, i, l]
    in_maps = []
    for k in range(N_CORES):
        wt_k = np.ascontiguousarray(
            np.transpose(W[k * OSH:(k + 1) * OSH], (2, 0, 1))  # [j, o, i]
        )
        in_maps.append({
            "wt": wt_k,
            "x1t": x1t,
            "x2": x2,
            "bias": b[k * OSH:(k + 1) * OSH].reshape(OSH, 1).copy(),
        })
    res = run_bass_kernel_spmd(nc, in_maps, core_ids=list(range(N_CORES)))
    outT = np.concatenate(
        [res.results[k]["out"] for k in range(N_CORES)], axis=1
    )  # [b, d3, l]
    return np.ascontiguousarray(np.transpose(outT, (0, 2, 1)))  # [b, l, d3]
